# revision 5
# baseline (speedup 1.0000x reference)
"""Involution2d Bass kernel for 8 trn2 NeuronCores.

Sharding: core = 2*b + half  (b = batch 0..3, half = group-half 0..1).
Each core computes out[b, half*128:(half+1)*128, :, :].

Math: ker = A @ x[b] + b_span  with A = w_span @ w_reduce folded on host
(rank-64 factorization folded; exact up to fp rounding).
out[c,p] = sum_kk ker[g(c),kk,p] * xpad[c, p+delta_kk]

Mapping:
 - ker-gen: PE matmuls (K=256 in 2 chunks), rows permuted kk-major (j = kk*8+g).
 - per tap kk: PE "replication" matmul (selection matrix) broadcasts the 8
   group-rows of tap kk to all 128 channel partitions (PSUM).
 - DVE tensor_tensor multiplies shifted xpad view by replicated ker (PSUM src).
 - PE identity matmuls accumulate the 49 tap products in PSUM.
"""
import numpy as np
from contextlib import ExitStack

B, C, H, W = 4, 256, 64, 64
G, K, PAD, R = 16, 7, 3, 4
HW = H * W
P = 128          # partitions / channels per core
NQ = 8           # pixel chunks
QPIX = HW // NQ  # 1024 pixels per quarter (16 image rows)
QROWS = H // NQ  # 16
JPAD = 512       # padded permuted-ker rows (392 -> 512)

_CACHE = {}


def _build_nc():
    import concourse.mybir as mybir
    import concourse.tile as tile
    from concourse import bacc

    f32 = mybir.dt.float32
    nc = bacc.Bacc("TRN2", target_bir_lowering=False, debug=False)

    xb = nc.dram_tensor("xb", (P, 2, H, W), f32, kind="ExternalInput")
    at = nc.dram_tensor("at", (P, 2, JPAD), f32, kind="ExternalInput")
    bias = nc.dram_tensor("bias", (P, 4), f32, kind="ExternalInput")
    bf16 = mybir.dt.bfloat16
    rep = nc.dram_tensor("rep", (P, 16, P), bf16, kind="ExternalInput")
    ident = nc.dram_tensor("ident", (P, P), mybir.dt.bfloat16, kind="ExternalInput")
    half_sel = nc.dram_tensor("half_sel", (P, 2), f32, kind="ExternalInput")
    out = nc.dram_tensor("out", (P, HW), f32, kind="ExternalOutput")

    with tile.TileContext(nc) as tc:
        with ExitStack() as ctx:
            const = ctx.enter_context(tc.tile_pool(name="const", bufs=1))
            ps_kg = ctx.enter_context(tc.tile_pool(name="ps_kg", bufs=1, space="PSUM"))
            ps_kerb = ctx.enter_context(tc.tile_pool(name="ps_kerb", bufs=4, space="PSUM"))
            ps_acc = ctx.enter_context(tc.tile_pool(name="ps_acc", bufs=2, space="PSUM"))
            sb_prod = ctx.enter_context(tc.tile_pool(name="sb_prod", bufs=4))
            sb_out = ctx.enter_context(tc.tile_pool(name="sb_out", bufs=2))

            x_sb = const.tile([P, 2, H, W], f32)
            at_sb = const.tile([P, 2, JPAD], f32)
            bias_sb = const.tile([P, 4], f32)
            rep_sb = const.tile([P, 16, P], bf16)
            id_sb = const.tile([P, P], bf16)
            hsel_sb = const.tile([P, 2], f32)
            ker_sb = const.tile([P, 4, HW], bf16)
            xpad7 = const.tile([P, K, H + 6, W], bf16)

            nc.sync.dma_start(x_sb[:], xb[:])
            nc.sync.dma_start(at_sb[:], at[:])
            nc.sync.dma_start(bias_sb[:], bias[:])
            nc.sync.dma_start(rep_sb[:], rep[:])
            nc.sync.dma_start(id_sb[:], ident[:])
            nc.sync.dma_start(hsel_sb[:], half_sel[:])

            # ---- xpad: zero border + our half's channels via PE select ----
            # x_half[c, :, :] = x_sb[:, half]; select via matmul with hsel?
            # Simpler: both halves' copies cost 2 ACT passes; select on host
            # instead: host sends xb with OUR half's 128 channels in slot 0.
            nc.vector.memset(xpad7[:], 0.0)
            for dj in range(K):
                s = dj - 3
                a, b = max(0, -s), min(W, W - s)
                nc.scalar.copy(
                    xpad7[:, dj, 3:3 + H, a:b],
                    x_sb[:, 0, :, a + s:b + s],
                )

            # ---- ker-gen: ker_sb[:, m, :] = (at[:, :, m-tile].T @ x) + bias ----
            for m in range(4):
                for n in range(8):
                    kg = ps_kg.tile([P, 512], f32)
                    for k in range(2):
                        nc.tensor.matmul(
                            kg[:],
                            at_sb[:, k, m * P:(m + 1) * P],
                            x_sb[:, k].rearrange("p h w -> p (h w)")[:, n * 512:(n + 1) * 512],
                            start=(k == 0), stop=(k == 1),
                        )
                    nc.scalar.add(
                        ker_sb[:, m, n * 512:(n + 1) * 512], kg[:],
                        bias_sb[:, m:m + 1],
                    )

            # ---- main loop: quarters x taps ----
            import concourse.mybir as _mb
            NT = K * K
            LOOKAHEAD = 2

            sb_kerb = ctx.enter_context(tc.tile_pool(name="sb_kerb", bufs=4))

            def emit_repl(q, kk):
                mt, tt = kk // 16, kk % 16
                kerb = ps_kerb.tile([P, QPIX], f32, tag="kerb")
                nc.tensor.matmul(
                    kerb[:],
                    rep_sb[:, tt, :],
                    ker_sb[:, mt, q * QPIX:(q + 1) * QPIX],
                    start=True, stop=True,
                )
                kerbS = sb_kerb.tile([P, QPIX], bf16, tag="kerbS")
                nc.scalar.copy(kerbS[:], kerb[:])
                return kerbS

            for q in range(NQ):
                acc = ps_acc.tile([P, QPIX], f32)
                r0 = q * QROWS
                kerbs = {kk: emit_repl(q, kk) for kk in range(LOOKAHEAD)}
                for kk in range(NT):
                    di, dj = kk // K, kk % K
                    prod = sb_prod.tile([P, QROWS, W], bf16)
                    nc.vector.tensor_tensor(
                        out=prod[:],
                        in0=xpad7[:, dj, di + r0: di + r0 + QROWS, :],
                        in1=kerbs.pop(kk)[:].rearrange("p (h w) -> p h w", w=W),
                        op=_mb.AluOpType.mult,
                    )
                    if kk + LOOKAHEAD < NT:
                        kerbs[kk + LOOKAHEAD] = emit_repl(q, kk + LOOKAHEAD)
                    nc.tensor.matmul(
                        acc[:],
                        id_sb[:],
                        prod[:].rearrange("p h w -> p (h w)"),
                        start=(kk == 0), stop=(kk == NT - 1),
                    )
                o_sb = sb_out.tile([P, QPIX], f32)
                nc.scalar.copy(o_sb[:], acc[:])
                nc.sync.dma_start(out[:, q * QPIX:(q + 1) * QPIX], o_sb[:])

    nc.compile()
    return nc


def _host_inputs(x, w_reduce, w_span, b_span):
    A = (w_span.astype(np.float64) @ w_reduce.astype(np.float64)).astype(np.float32)
    import ml_dtypes as _md
    ident = np.eye(P, dtype=_md.bfloat16)
    rep = np.zeros((P, 16, P), dtype=np.float32)
    for p in range(P):
        for m in range(P):
            t = p // 8
            if p == t * 8 + m // 16:
                rep[p, t, m] = 1.0
    # rep[p, t, m] = 1 iff p == t*8 + m//16
    import ml_dtypes
    rep = np.zeros((P, 16, P), dtype=np.float32)
    for t in range(16):
        for m in range(P):
            rep[t * 8 + m // 16, t, m] = 1.0
    rep = rep.astype(ml_dtypes.bfloat16)

    in_maps = []
    for core in range(8):
        b, half = core // 2, core % 2
        # permuted fold: j = kk*8 + g  ->  A row (half*8+g)*49 + kk
        Ap = np.zeros((JPAD, C), dtype=np.float32)
        bp = np.zeros((JPAD,), dtype=np.float32)
        for kk in range(K * K):
            for g in range(8):
                j = kk * 8 + g
                src = (half * 8 + g) * (K * K) + kk
                Ap[j] = A[src]
                bp[j] = b_span[src]
        at = np.ascontiguousarray(
            Ap.T.reshape(2, P, JPAD).transpose(1, 0, 2))  # [P, 2, JPAD]
        bias = np.ascontiguousarray(bp.reshape(4, P).T)   # [P, 4]
        xh = x[b, half * P:(half + 1) * P]                # [128, H, W] our half
        xo = x[b, (1 - half) * P:(2 - half) * P]          # other half
        xb_arr = np.stack([xh, xo], axis=1)               # [P, 2, H, W]
        # ker-gen contracts over channel chunks k=0 (rows 0..127) and k=1:
        # chunk k must hold x channels k*128..k*128+127 in ORIGINAL order.
        # With xb[:,0]=our half, xb[:,1]=other: the A columns must be permuted
        # to match: columns [half*128:(half+1)*128] first, then the rest.
        colperm = np.concatenate([
            np.arange(half * P, (half + 1) * P),
            np.arange((1 - half) * P, (2 - half) * P)])
        Ap2 = Ap[:, colperm]
        at = np.ascontiguousarray(
            Ap2.T.reshape(2, P, JPAD).transpose(1, 0, 2))
        hsel = np.zeros((P, 2), dtype=np.float32)
        hsel[:, 0] = 1.0
        in_maps.append({
            "xb": np.ascontiguousarray(xb_arr, dtype=np.float32),
            "at": at.astype(np.float32),
            "bias": bias.astype(np.float32),
            "rep": rep,
            "ident": ident,
            "half_sel": hsel,
        })
    return in_maps


def kernel(x, w_reduce, w_span, b_span):
    from concourse import bass_utils
    x = np.asarray(x, dtype=np.float32)
    w_reduce = np.asarray(w_reduce, dtype=np.float32)
    w_span = np.asarray(w_span, dtype=np.float32)
    b_span = np.asarray(b_span, dtype=np.float32)

    if "nc" not in _CACHE:
        _CACHE["nc"] = _build_nc()
    nc = _CACHE["nc"]

    in_maps = _host_inputs(x, w_reduce, w_span, b_span)
    res = bass_utils.run_bass_kernel_spmd(nc, in_maps, core_ids=list(range(8)))

    out = np.empty((B, C, H, W), dtype=np.float32)
    for core in range(8):
        b, half = core // 2, core % 2
        out[b, half * P:(half + 1) * P] = res.results[core]["out"].reshape(P, H, W)
    return out


# revision 7
# speedup vs baseline: 1.1102x; 1.1102x over previous
"""Involution2d Bass kernel for 8 trn2 NeuronCores.

Sharding: core = 2*b + half  (b = batch 0..3, half = group-half 0..1).
Each core computes out[b, half*128:(half+1)*128, :, :].

Math: ker = A @ x[b] + b_span  with A = w_span @ w_reduce folded on host
(rank-64 factorization folded; exact up to fp rounding).
out[c,p] = sum_kk ker[g(c),kk,p] * xpad[c, p+delta_kk]

Mapping:
 - ker-gen: PE matmuls (K=256 in 2 chunks), rows permuted kk-major (j = kk*8+g).
 - per tap kk: PE "replication" matmul (selection matrix) broadcasts the 8
   group-rows of tap kk to all 128 channel partitions (PSUM).
 - DVE tensor_tensor multiplies shifted xpad view by replicated ker (PSUM src).
 - PE identity matmuls accumulate the 49 tap products in PSUM.
"""
import numpy as np
from contextlib import ExitStack

B, C, H, W = 4, 256, 64, 64
G, K, PAD, R = 16, 7, 3, 4
HW = H * W
P = 128          # partitions / channels per core
NQ = 4           # pixel chunks
QPIX = HW // NQ  # 1024 pixels per quarter (16 image rows)
QROWS = H // NQ  # 16
JPAD = 512       # padded permuted-ker rows (392 -> 512)

_CACHE = {}


def _build_nc():
    import concourse.mybir as mybir
    import concourse.tile as tile
    from concourse import bacc

    f32 = mybir.dt.float32
    nc = bacc.Bacc("TRN2", target_bir_lowering=False, debug=False)

    xb = nc.dram_tensor("xb", (P, 2, H, W), f32, kind="ExternalInput")
    at = nc.dram_tensor("at", (P, 2, JPAD), f32, kind="ExternalInput")
    bias = nc.dram_tensor("bias", (P, 4), f32, kind="ExternalInput")
    bf16 = mybir.dt.bfloat16
    rep = nc.dram_tensor("rep", (P, 16, P), bf16, kind="ExternalInput")
    ident = nc.dram_tensor("ident", (P, P), mybir.dt.bfloat16, kind="ExternalInput")
    half_sel = nc.dram_tensor("half_sel", (P, 2), f32, kind="ExternalInput")
    out = nc.dram_tensor("out", (P, HW), f32, kind="ExternalOutput")

    with tile.TileContext(nc) as tc:
        with ExitStack() as ctx:
            const = ctx.enter_context(tc.tile_pool(name="const", bufs=1))
            ps_kg = ctx.enter_context(tc.tile_pool(name="ps_kg", bufs=1, space="PSUM"))
            ps_kerb = ctx.enter_context(tc.tile_pool(name="ps_kerb", bufs=2, space="PSUM"))
            ps_acc = ctx.enter_context(tc.tile_pool(name="ps_acc", bufs=1, space="PSUM"))
            sb_prod = ctx.enter_context(tc.tile_pool(name="sb_prod", bufs=4))
            sb_out = ctx.enter_context(tc.tile_pool(name="sb_out", bufs=2))

            x_sb = const.tile([P, 2, H, W], f32)
            at_sb = const.tile([P, 2, JPAD], f32)
            bias_sb = const.tile([P, 4], f32)
            rep_sb = const.tile([P, 16, P], bf16)
            id_sb = const.tile([P, P], bf16)
            hsel_sb = const.tile([P, 2], f32)
            ker_sb = const.tile([P, 4, HW], bf16)
            xpad7 = const.tile([P, K, H + 6, W], bf16)

            nc.sync.dma_start(x_sb[:], xb[:])
            nc.sync.dma_start(at_sb[:], at[:])
            nc.sync.dma_start(bias_sb[:], bias[:])
            nc.sync.dma_start(rep_sb[:], rep[:])
            nc.sync.dma_start(id_sb[:], ident[:])
            nc.sync.dma_start(hsel_sb[:], half_sel[:])

            # ---- xpad: zero border + our half's channels via PE select ----
            # x_half[c, :, :] = x_sb[:, half]; select via matmul with hsel?
            # Simpler: both halves' copies cost 2 ACT passes; select on host
            # instead: host sends xb with OUR half's 128 channels in slot 0.
            nc.vector.memset(xpad7[:], 0.0)
            for dj in range(K):
                s = dj - 3
                a, b = max(0, -s), min(W, W - s)
                nc.scalar.copy(
                    xpad7[:, dj, 3:3 + H, a:b],
                    x_sb[:, 0, :, a + s:b + s],
                )

            # ---- ker-gen: ker_sb[:, m, :] = (at[:, :, m-tile].T @ x) + bias ----
            for m in range(4):
                for n in range(8):
                    kg = ps_kg.tile([P, 512], f32)
                    for k in range(2):
                        nc.tensor.matmul(
                            kg[:],
                            at_sb[:, k, m * P:(m + 1) * P],
                            x_sb[:, k].rearrange("p h w -> p (h w)")[:, n * 512:(n + 1) * 512],
                            start=(k == 0), stop=(k == 1),
                        )
                    nc.scalar.add(
                        ker_sb[:, m, n * 512:(n + 1) * 512], kg[:],
                        bias_sb[:, m:m + 1],
                    )

            # ---- main loop: quarters x taps ----
            import concourse.mybir as _mb
            NT = K * K
            LOOKAHEAD = 2

            sb_kerb = ctx.enter_context(tc.tile_pool(name="sb_kerb", bufs=4))

            def emit_repl(q, kk):
                mt, tt = kk // 16, kk % 16
                kerb = ps_kerb.tile([P, QPIX], f32, tag="kerb")
                rg = 32 * ((tt % 16) // 4)
                for hh in range(2):
                    nc.tensor.matmul(
                        kerb[:, hh * 512:(hh + 1) * 512],
                        rep_sb[rg:rg + 32, tt, :],
                        ker_sb[rg:rg + 32, mt, q * QPIX + hh * 512:q * QPIX + (hh + 1) * 512],
                        start=True, stop=True,
                        tile_position=(rg, 0),
                    )
                kerbS = sb_kerb.tile([P, QPIX], bf16, tag="kerbS")
                nc.scalar.copy(kerbS[:], kerb[:])
                return kerbS

            for q in range(NQ):
                acc = ps_acc.tile([P, QPIX], f32)
                r0 = q * QROWS
                buckets = [[kk for kk in range(NT) if ((kk % 16) // 4) == r]
                           for r in range(4)]
                order = []
                while any(buckets):
                    for bkt in buckets:
                        if bkt:
                            order.append(bkt.pop(0))
                kerbs = {kk: emit_repl(q, kk) for kk in order[:LOOKAHEAD]}
                for i, kk in enumerate(order):
                    di, dj = kk // K, kk % K
                    prod = sb_prod.tile([P, QROWS, W], bf16)
                    nc.vector.tensor_tensor(
                        out=prod[:],
                        in0=xpad7[:, dj, di + r0: di + r0 + QROWS, :],
                        in1=kerbs.pop(kk)[:].rearrange("p (h w) -> p h w", w=W),
                        op=_mb.AluOpType.mult,
                    )
                    if i + LOOKAHEAD < NT:
                        nkk = order[i + LOOKAHEAD]
                        kerbs[nkk] = emit_repl(q, nkk)
                    pr = prod[:].rearrange("p h w -> p (h w)")
                    for hh in range(2):
                        nc.tensor.matmul(
                            acc[:, hh * 512:(hh + 1) * 512],
                            id_sb[:],
                            pr[:, hh * 512:(hh + 1) * 512],
                            start=(i == 0), stop=(i == NT - 1),
                        )
                o_sb = sb_out.tile([P, QPIX], f32)
                nc.scalar.copy(o_sb[:], acc[:])
                nc.sync.dma_start(out[:, q * QPIX:(q + 1) * QPIX], o_sb[:])

    nc.compile()
    return nc


def _host_inputs(x, w_reduce, w_span, b_span):
    A = (w_span.astype(np.float64) @ w_reduce.astype(np.float64)).astype(np.float32)
    import ml_dtypes as _md
    ident = np.eye(P, dtype=_md.bfloat16)
    rep = np.zeros((P, 16, P), dtype=np.float32)
    for p in range(P):
        for m in range(P):
            t = p // 8
            if p == t * 8 + m // 16:
                rep[p, t, m] = 1.0
    # rep[p, t, m] = 1 iff p == t*8 + m//16
    import ml_dtypes
    rep = np.zeros((P, 16, P), dtype=np.float32)
    for t in range(16):
        for m in range(P):
            rep[t * 8 + m // 16, t, m] = 1.0
    rep = rep.astype(ml_dtypes.bfloat16)

    in_maps = []
    for core in range(8):
        b, half = core // 2, core % 2
        # permuted fold: j = kk*8 + g  ->  A row (half*8+g)*49 + kk
        Ap = np.zeros((JPAD, C), dtype=np.float32)
        bp = np.zeros((JPAD,), dtype=np.float32)
        for kk in range(K * K):
            for g in range(8):
                j = kk * 8 + g
                src = (half * 8 + g) * (K * K) + kk
                Ap[j] = A[src]
                bp[j] = b_span[src]
        at = np.ascontiguousarray(
            Ap.T.reshape(2, P, JPAD).transpose(1, 0, 2))  # [P, 2, JPAD]
        bias = np.ascontiguousarray(bp.reshape(4, P).T)   # [P, 4]
        xh = x[b, half * P:(half + 1) * P]                # [128, H, W] our half
        xo = x[b, (1 - half) * P:(2 - half) * P]          # other half
        xb_arr = np.stack([xh, xo], axis=1)               # [P, 2, H, W]
        # ker-gen contracts over channel chunks k=0 (rows 0..127) and k=1:
        # chunk k must hold x channels k*128..k*128+127 in ORIGINAL order.
        # With xb[:,0]=our half, xb[:,1]=other: the A columns must be permuted
        # to match: columns [half*128:(half+1)*128] first, then the rest.
        colperm = np.concatenate([
            np.arange(half * P, (half + 1) * P),
            np.arange((1 - half) * P, (2 - half) * P)])
        Ap2 = Ap[:, colperm]
        at = np.ascontiguousarray(
            Ap2.T.reshape(2, P, JPAD).transpose(1, 0, 2))
        hsel = np.zeros((P, 2), dtype=np.float32)
        hsel[:, 0] = 1.0
        in_maps.append({
            "xb": np.ascontiguousarray(xb_arr, dtype=np.float32),
            "at": at.astype(np.float32),
            "bias": bias.astype(np.float32),
            "rep": rep,
            "ident": ident,
            "half_sel": hsel,
        })
    return in_maps


def kernel(x, w_reduce, w_span, b_span):
    from concourse import bass_utils
    x = np.asarray(x, dtype=np.float32)
    w_reduce = np.asarray(w_reduce, dtype=np.float32)
    w_span = np.asarray(w_span, dtype=np.float32)
    b_span = np.asarray(b_span, dtype=np.float32)

    if "nc" not in _CACHE:
        _CACHE["nc"] = _build_nc()
    nc = _CACHE["nc"]

    in_maps = _host_inputs(x, w_reduce, w_span, b_span)
    res = bass_utils.run_bass_kernel_spmd(nc, in_maps, core_ids=list(range(8)))

    out = np.empty((B, C, H, W), dtype=np.float32)
    for core in range(8):
        b, half = core // 2, core % 2
        out[b, half * P:(half + 1) * P] = res.results[core]["out"].reshape(P, H, W)
    return out
